# revision 41
# baseline (speedup 1.0000x reference)
"""GaussianNB log-posterior kernel for 8 Trainium2 NeuronCores.

out[b, c] = log_pi[c] - 0.5 * sum_f(log2pi + log_var[c,f] + (x[b,f]-mu[c,f])^2 / var[c,f])
          = const[c] + sum_f wq[c,f]*x[b,f]^2 + wc[c,f]*x[b,f]
  with wq = -0.5*exp(-log_var), wc = mu*exp(-log_var),
       const = log_pi - 0.5*(F*log2pi + sum_f log_var + sum_f mu^2*exp(-log_var)).

Strategy: data-parallel over batch (B=2048 -> 256 rows/core); weights
replicated. All layout work (transpose to f-major, SBUF-layout packing,
fp8 cast) and the O((B+C)F) elementwise weight prep happen on host; the
device does the O(B*F*C) GEMMs in fp8e4 DoubleRow mode (2 k-tiles per
matmul), accumulating fp32 in PSUM, then adds const[c] and DMAs out.

Two variants, picked per call:
 - general: outT = wcT*xT + wqT*x2T  (16 DoubleRow matmuls)
 - log_var constant across (c,f) (e.g. all zeros): wq[c,f] == wq0, so
   the quad term collapses to the rank-1 update ones[c] * (wq0*sum_f
   x[b,f]^2).  Host sends q[b] = wq0*sum_f x2 as one fp32 contraction
   row; the x2/wq chunk (half the input bytes) and its 8 matmuls
   disappear.

A few dummy matmuls on scratch SBUF run while the DMAs stream in,
keeping the PE busy so the HAM clock gate opens to 2.4 GHz.
"""
import sys

sys.path.insert(0, "/opt/trn_rl_repo")
import numpy as np
import concourse.bacc as bacc
import concourse.mybir as mybir
from concourse.tile import TileContext
from concourse.bass_utils import run_bass_kernel_spmd

B, C, F = 2048, 256, 1024
NCORES = 8
BSH = B // NCORES  # 256
KT = F // 128      # 8 k-tiles
LOG_2PI = float(np.log(2.0 * np.pi))
F32 = mybir.dt.float32
BF16 = mybir.dt.bfloat16
FP16 = mybir.dt.float16
FP8 = mybir.dt.float8e4
NPFP8 = mybir.dt.np(FP8)
DR = mybir.MatmulPerfMode.DoubleRow
WARM_MMS = 0
CHUNKS = (4, 2, 2)  # k-tiles per input chunk (need-ordered, serialized queue)

_CACHE = {}


def _build(rank1: bool):
    nc = bacc.Bacc("TRN2", target_bir_lowering=False, debug=False, num_devices=NCORES)
    # Host-packed fp8 chunks, f-major SBUF layout (dim1 = k-tile index).
    # Chunk j covers CHUNKS[j] k-tiles: first its xT k-tiles (r=b), then
    # the matching wcT k-tiles (r=c).  Chunks are DMA'd in need order on
    # one queue so matmuls start as soon as the first chunk lands and
    # only two matmuls remain after the last input byte.
    ach_d = [
        nc.dram_tensor(f"a{j}", [128, 2 * kk, 256], FP8, kind="ExternalInput").ap()
        for j, kk in enumerate(CHUNKS)
    ]
    if rank1:
        # aux row0: q[b] = wq0*sum_f x[b,f]^2 | ones[c]
        # aux row1: ones[b]                   | const[c]
        # K=2 rank-2 matmul adds both the quad term and const[c], so no
        # separate cst DMA or epilogue add is needed on this path.
        aux_d = nc.dram_tensor("aux", [2, 512], FP16, kind="ExternalInput").ap()
    else:
        # a2[p, i, r]: i 0..7 -> wqT k 0..7 (r=c), i 8..15 -> x2T k 0..7 (r=b)
        a2_d = nc.dram_tensor("a2", [128, 16, 256], FP8, kind="ExternalInput").ap()
        cst_d = nc.dram_tensor("cst", [128, 2], F32, kind="ExternalInput").ap()
    # out[p, 256m+b] = outT[128m+p, b]
    out_d = nc.dram_tensor("out", [128, 2 * BSH], F32, kind="ExternalOutput").ap()

    with TileContext(nc) as tc:
        with (
            tc.tile_pool(name="sb", bufs=1) as sb,
            tc.tile_pool(name="pg", bufs=2, space="PSUM") as pgp,
            tc.tile_pool(name="pw", bufs=1, space="PSUM") as pwp,
        ):
            # ---- PE warmup on scratch (result never read) while DMAs stream ----
            if WARM_MMS:
                junk = sb.tile([128, 512], BF16, tag="junk")
                wps = pwp.tile([128, 512], F32, tag="wps")
                nc.vector.memset(junk[:], 0)
                for _ in range(WARM_MMS):
                    nc.tensor.matmul(wps[:], junk[:, :128], junk[:], start=True, stop=True)

            # ---- DMA in (HWDGE via sync + scalar queues) ----
            # all x|wc chunks on the same (sync) queue: serialized in need
            # order, so early chunks complete while later ones stream.
            ach = [
                sb.tile([128, 2 * kk, 256], FP8, tag=f"a{j}", name=f"a{j}")
                for j, kk in enumerate(CHUNKS)
            ]
            for t_sb, t_d in zip(ach, ach_d):
                nc.sync.dma_start(out=t_sb[:], in_=t_d[:])
            if rank1:
                aux = sb.tile([2, 512], FP16, tag="aux")
                nc.scalar.dma_start(out=aux[:], in_=aux_d[:])
            else:
                a2 = sb.tile([128, 16, 256], FP8, tag="a2")
                nc.scalar.dma_start(out=a2[:, 0:8, :], in_=a2_d[:, 0:8, :])
                nc.scalar.dma_start(out=a2[:, 8:16, :], in_=a2_d[:, 8:16, :])
                cst = sb.tile([128, 2], F32, tag="cst")
                nc.scalar.dma_start(out=cst[:], in_=cst_d[:])

            # ---- GEMM: outT[c,b], fp8 DoubleRow (2 k-tiles per matmul) ----
            pg = [pgp.tile([128, BSH], F32, tag=f"pg{m}", name=f"pg{m}") for m in range(2)]
            started = [False, False]

            def mm(m, w_tile, w_i, a_tile, a_i, stop=False):
                nc.tensor.matmul(
                    pg[m][:],
                    w_tile[:, w_i:w_i + 2, m * 128:(m + 1) * 128],
                    a_tile[:, a_i:a_i + 2, :],
                    start=not started[m],
                    stop=stop,
                    perf_mode=DR,
                )
                started[m] = True

            def chunk_mms(j, stop=False):
                kk = CHUNKS[j]
                for t in range(kk // 2):
                    for m in range(2):
                        mm(m, ach[j], kk + 2 * t, ach[j], 2 * t,
                           stop=(stop and t == kk // 2 - 1))

            chunk_mms(0)
            if rank1:
                # += ones[c]*q[b] + const[c]*ones[b] (K=2).  aux's
                # descriptors drain behind chunk 0's at the SDMA engines,
                # so it lands just after — slot these here.
                for m in range(2):
                    nc.tensor.matmul(
                        pg[m][:],
                        aux[:, 256 + m * 128: 256 + m * 128 + 128],
                        aux[:, 0:256],
                        start=False,
                        stop=False,
                    )
            for j in range(1, len(CHUNKS)):
                chunk_mms(j, stop=(rank1 and j == len(CHUNKS) - 1))
            if not rank1:
                for m in range(2):       # wq * x2, k-tiles 0..7 (chunk a2)
                    for t in range(4):
                        mm(m, a2, 2 * t, a2, 8 + 2 * t, stop=(t == 3))

            # ---- epilogue: PSUM -> SBUF (+const on the general path), DMA out ----
            for m in range(2):
                osb = sb.tile([128, BSH], F32, tag=f"os{m}", name=f"os{m}")
                if rank1:
                    nc.vector.tensor_copy(osb[:], pg[m][:])
                else:
                    nc.vector.tensor_scalar_add(osb[:], pg[m][:], cst[:, m:m + 1])
                eng = nc.sync if m == 0 else nc.scalar
                eng.dma_start(out=out_d[:, m * BSH:(m + 1) * BSH], in_=osb[:])

    nc.compile()
    return nc


def get_nc(rank1=True):
    key = f"nc{int(bool(rank1))}"
    if key not in _CACHE:
        _CACHE[key] = _build(rank1)
    return _CACHE[key]


def _pack_fmajor(a):
    """(R, F) f32 -> [128, KT, R] fp8 with out[p, k, r] = a[r, 128k+p]."""
    R = a.shape[0]
    return a.reshape(R, KT, 128).transpose(2, 1, 0).astype(NPFP8)


def prepare_in_maps(x, mu, log_var, log_pi, force_general=False):
    x = np.asarray(x, dtype=np.float32)
    mu = np.asarray(mu, dtype=np.float32)
    lv = np.asarray(log_var, dtype=np.float32)
    lp = np.asarray(log_pi, dtype=np.float32)

    inv = np.exp(-lv)
    wc = mu * inv
    const = lp - 0.5 * (F * LOG_2PI + lv.sum(axis=1) + (mu * mu * inv).sum(axis=1))
    rank1 = bool(np.ptp(lv) == 0.0) and not force_general

    wcp = _pack_fmajor(wc)                      # [128, 8, 256]
    if not rank1:
        wqp = _pack_fmajor(-0.5 * inv)
        cst = np.ascontiguousarray(const.reshape(2, 128).T.astype(np.float32))
    else:
        wq0 = -0.5 * float(np.exp(-lv.flat[0]))

    maps = []
    for c in range(NCORES):
        xs = x[c * BSH:(c + 1) * BSH]
        xp = _pack_fmajor(xs)
        m = {}
        k0 = 0
        for j, kk in enumerate(CHUNKS):
            m[f"a{j}"] = np.ascontiguousarray(
                np.concatenate([xp[:, k0:k0 + kk], wcp[:, k0:k0 + kk]], axis=1))
            k0 += kk
        if rank1:
            aux = np.zeros((2, 512), dtype=np.float16)
            aux[0, 0:256] = (wq0 * (xs.astype(np.float64) ** 2).sum(axis=1)).astype(np.float16)
            aux[0, 256:512] = 1.0
            aux[1, 0:256] = 1.0
            aux[1, 256:512] = const.astype(np.float16)
            m["aux"] = aux
        else:
            m["a2"] = np.ascontiguousarray(
                np.concatenate([wqp, _pack_fmajor(xs * xs)], axis=1))
            m["cst"] = cst
        maps.append(m)
    return maps, rank1


def unpack_out(results):
    out = np.empty((B, C), dtype=np.float32)
    for c in range(NCORES):
        res = results[c]["out"].astype(np.float32)  # [128, 2*BSH] bf16 -> f32
        v = res.reshape(128, 2, BSH)
        out[c * BSH:(c + 1) * BSH, :] = v.transpose(2, 1, 0).reshape(BSH, C)
    return out


def kernel(x, mu, log_var, log_pi):
    in_maps, rank1 = prepare_in_maps(x, mu, log_var, log_pi)
    nc = get_nc(rank1)
    res = run_bass_kernel_spmd(nc, in_maps, list(range(NCORES)))
    return unpack_out(res.results)


# revision 42
# speedup vs baseline: 1.1284x; 1.1284x over previous
"""GaussianNB log-posterior kernel for 8 Trainium2 NeuronCores.

out[b, c] = log_pi[c] - 0.5 * sum_f(log2pi + log_var[c,f] + (x[b,f]-mu[c,f])^2 / var[c,f])
          = const[c] + sum_f wq[c,f]*x[b,f]^2 + wc[c,f]*x[b,f]
  with wq = -0.5*exp(-log_var), wc = mu*exp(-log_var),
       const = log_pi - 0.5*(F*log2pi + sum_f log_var + sum_f mu^2*exp(-log_var)).

Strategy: data-parallel over batch (B=2048 -> 256 rows/core); weights
replicated. All layout work (transpose to f-major, SBUF-layout packing,
fp8 cast) and the O((B+C)F) elementwise weight prep happen on host; the
device does the O(B*F*C) GEMMs in fp8e4 DoubleRow mode (2 k-tiles per
matmul), accumulating fp32 in PSUM, then adds const[c] and DMAs out.

Two variants, picked per call:
 - general: outT = wcT*xT + wqT*x2T  (16 DoubleRow matmuls)
 - log_var constant across (c,f) (e.g. all zeros): wq[c,f] == wq0, so
   the quad term collapses to the rank-1 update ones[c] * (wq0*sum_f
   x[b,f]^2).  Host sends q[b] = wq0*sum_f x2 as one fp32 contraction
   row; the x2/wq chunk (half the input bytes) and its 8 matmuls
   disappear.

A few dummy matmuls on scratch SBUF run while the DMAs stream in,
keeping the PE busy so the HAM clock gate opens to 2.4 GHz.
"""
import sys

sys.path.insert(0, "/opt/trn_rl_repo")
import numpy as np
import concourse.bacc as bacc
import concourse.mybir as mybir
from concourse.tile import TileContext
from concourse.bass_utils import run_bass_kernel_spmd

B, C, F = 2048, 256, 1024
NCORES = 8
BSH = B // NCORES  # 256
KT = F // 128      # 8 k-tiles
LOG_2PI = float(np.log(2.0 * np.pi))
F32 = mybir.dt.float32
BF16 = mybir.dt.bfloat16
FP16 = mybir.dt.float16
FP8 = mybir.dt.float8e4
NPFP8 = mybir.dt.np(FP8)
DR = mybir.MatmulPerfMode.DoubleRow
WARM_MMS = 0
CHUNKS = (6, 2)  # k-tiles per input chunk (need-ordered, serialized queue)

_CACHE = {}


def _build(rank1: bool):
    nc = bacc.Bacc("TRN2", target_bir_lowering=False, debug=False, num_devices=NCORES)
    # Host-packed fp8 chunks, f-major SBUF layout (dim1 = k-tile index).
    # Chunk j covers CHUNKS[j] k-tiles: first its xT k-tiles (r=b), then
    # the matching wcT k-tiles (r=c).  Chunks are DMA'd in need order on
    # one queue so matmuls start as soon as the first chunk lands and
    # only two matmuls remain after the last input byte.
    ach_d = [
        nc.dram_tensor(f"a{j}", [128, 2 * kk, 256], FP8, kind="ExternalInput").ap()
        for j, kk in enumerate(CHUNKS)
    ]
    if rank1:
        # aux row0: q[b] = wq0*sum_f x[b,f]^2 | ones[c]
        # aux row1: ones[b]                   | const[c]
        # K=2 rank-2 matmul adds both the quad term and const[c], so no
        # separate cst DMA or epilogue add is needed on this path.
        aux_d = nc.dram_tensor("aux", [2, 512], FP16, kind="ExternalInput").ap()
    else:
        # a2[p, i, r]: i 0..7 -> wqT k 0..7 (r=c), i 8..15 -> x2T k 0..7 (r=b)
        a2_d = nc.dram_tensor("a2", [128, 16, 256], FP8, kind="ExternalInput").ap()
        cst_d = nc.dram_tensor("cst", [128, 2], F32, kind="ExternalInput").ap()
    # out[p, 256m+b] = outT[128m+p, b]
    out_d = nc.dram_tensor("out", [128, 2 * BSH], F32, kind="ExternalOutput").ap()

    with TileContext(nc) as tc:
        with (
            tc.tile_pool(name="sb", bufs=1) as sb,
            tc.tile_pool(name="pg", bufs=2, space="PSUM") as pgp,
            tc.tile_pool(name="pw", bufs=1, space="PSUM") as pwp,
        ):
            # ---- PE warmup on scratch (result never read) while DMAs stream ----
            if WARM_MMS:
                junk = sb.tile([128, 512], BF16, tag="junk")
                wps = pwp.tile([128, 512], F32, tag="wps")
                nc.vector.memset(junk[:], 0)
                for _ in range(WARM_MMS):
                    nc.tensor.matmul(wps[:], junk[:, :128], junk[:], start=True, stop=True)

            # ---- DMA in (HWDGE via sync + scalar queues) ----
            # all x|wc chunks on the same (sync) queue: serialized in need
            # order, so early chunks complete while later ones stream.
            ach = [
                sb.tile([128, 2 * kk, 256], FP8, tag=f"a{j}", name=f"a{j}")
                for j, kk in enumerate(CHUNKS)
            ]
            for t_sb, t_d in zip(ach, ach_d):
                nc.sync.dma_start(out=t_sb[:], in_=t_d[:])
            if rank1:
                aux = sb.tile([2, 512], FP16, tag="aux")
                nc.scalar.dma_start(out=aux[:], in_=aux_d[:])
            else:
                a2 = sb.tile([128, 16, 256], FP8, tag="a2")
                nc.scalar.dma_start(out=a2[:, 0:8, :], in_=a2_d[:, 0:8, :])
                nc.scalar.dma_start(out=a2[:, 8:16, :], in_=a2_d[:, 8:16, :])
                cst = sb.tile([128, 2], F32, tag="cst")
                nc.scalar.dma_start(out=cst[:], in_=cst_d[:])

            # ---- GEMM: outT[c,b], fp8 DoubleRow (2 k-tiles per matmul) ----
            pg = [pgp.tile([128, BSH], F32, tag=f"pg{m}", name=f"pg{m}") for m in range(2)]
            started = [False, False]

            def mm(m, w_tile, w_i, a_tile, a_i, stop=False):
                nc.tensor.matmul(
                    pg[m][:],
                    w_tile[:, w_i:w_i + 2, m * 128:(m + 1) * 128],
                    a_tile[:, a_i:a_i + 2, :],
                    start=not started[m],
                    stop=stop,
                    perf_mode=DR,
                )
                started[m] = True

            def chunk_mms(j, stop=False):
                kk = CHUNKS[j]
                for t in range(kk // 2):
                    for m in range(2):
                        mm(m, ach[j], kk + 2 * t, ach[j], 2 * t,
                           stop=(stop and t == kk // 2 - 1))

            chunk_mms(0)
            if rank1:
                # += ones[c]*q[b] + const[c]*ones[b] (K=2).  aux's
                # descriptors drain behind chunk 0's at the SDMA engines,
                # so it lands just after — slot these here.
                for m in range(2):
                    nc.tensor.matmul(
                        pg[m][:],
                        aux[:, 256 + m * 128: 256 + m * 128 + 128],
                        aux[:, 0:256],
                        start=False,
                        stop=False,
                    )
            for j in range(1, len(CHUNKS)):
                chunk_mms(j, stop=(rank1 and j == len(CHUNKS) - 1))
            if not rank1:
                for m in range(2):       # wq * x2, k-tiles 0..7 (chunk a2)
                    for t in range(4):
                        mm(m, a2, 2 * t, a2, 8 + 2 * t, stop=(t == 3))

            # ---- epilogue: PSUM -> SBUF (+const on the general path), DMA out ----
            for m in range(2):
                osb = sb.tile([128, BSH], F32, tag=f"os{m}", name=f"os{m}")
                if rank1:
                    nc.vector.tensor_copy(osb[:], pg[m][:])
                else:
                    nc.vector.tensor_scalar_add(osb[:], pg[m][:], cst[:, m:m + 1])
                eng = nc.sync if m == 0 else nc.scalar
                eng.dma_start(out=out_d[:, m * BSH:(m + 1) * BSH], in_=osb[:])

    nc.compile()
    return nc


def get_nc(rank1=True):
    key = f"nc{int(bool(rank1))}"
    if key not in _CACHE:
        _CACHE[key] = _build(rank1)
    return _CACHE[key]


def _pack_fmajor(a):
    """(R, F) f32 -> [128, KT, R] fp8 with out[p, k, r] = a[r, 128k+p]."""
    R = a.shape[0]
    return a.reshape(R, KT, 128).transpose(2, 1, 0).astype(NPFP8)


def prepare_in_maps(x, mu, log_var, log_pi, force_general=False):
    x = np.asarray(x, dtype=np.float32)
    mu = np.asarray(mu, dtype=np.float32)
    lv = np.asarray(log_var, dtype=np.float32)
    lp = np.asarray(log_pi, dtype=np.float32)

    inv = np.exp(-lv)
    wc = mu * inv
    const = lp - 0.5 * (F * LOG_2PI + lv.sum(axis=1) + (mu * mu * inv).sum(axis=1))
    rank1 = bool(np.ptp(lv) == 0.0) and not force_general

    wcp = _pack_fmajor(wc)                      # [128, 8, 256]
    if not rank1:
        wqp = _pack_fmajor(-0.5 * inv)
        cst = np.ascontiguousarray(const.reshape(2, 128).T.astype(np.float32))
    else:
        wq0 = -0.5 * float(np.exp(-lv.flat[0]))

    maps = []
    for c in range(NCORES):
        xs = x[c * BSH:(c + 1) * BSH]
        xp = _pack_fmajor(xs)
        m = {}
        k0 = 0
        for j, kk in enumerate(CHUNKS):
            m[f"a{j}"] = np.ascontiguousarray(
                np.concatenate([xp[:, k0:k0 + kk], wcp[:, k0:k0 + kk]], axis=1))
            k0 += kk
        if rank1:
            aux = np.zeros((2, 512), dtype=np.float16)
            aux[0, 0:256] = (wq0 * (xs.astype(np.float64) ** 2).sum(axis=1)).astype(np.float16)
            aux[0, 256:512] = 1.0
            aux[1, 0:256] = 1.0
            aux[1, 256:512] = const.astype(np.float16)
            m["aux"] = aux
        else:
            m["a2"] = np.ascontiguousarray(
                np.concatenate([wqp, _pack_fmajor(xs * xs)], axis=1))
            m["cst"] = cst
        maps.append(m)
    return maps, rank1


def unpack_out(results):
    out = np.empty((B, C), dtype=np.float32)
    for c in range(NCORES):
        res = results[c]["out"].astype(np.float32)  # [128, 2*BSH] bf16 -> f32
        v = res.reshape(128, 2, BSH)
        out[c * BSH:(c + 1) * BSH, :] = v.transpose(2, 1, 0).reshape(BSH, C)
    return out


def kernel(x, mu, log_var, log_pi):
    in_maps, rank1 = prepare_in_maps(x, mu, log_var, log_pi)
    nc = get_nc(rank1)
    res = run_bass_kernel_spmd(nc, in_maps, list(range(NCORES)))
    return unpack_out(res.results)


# revision 43
# speedup vs baseline: 1.1334x; 1.0044x over previous
"""GaussianNB log-posterior kernel for 8 Trainium2 NeuronCores.

out[b, c] = log_pi[c] - 0.5 * sum_f(log2pi + log_var[c,f] + (x[b,f]-mu[c,f])^2 / var[c,f])
          = const[c] + sum_f wq[c,f]*x[b,f]^2 + wc[c,f]*x[b,f]
  with wq = -0.5*exp(-log_var), wc = mu*exp(-log_var),
       const = log_pi - 0.5*(F*log2pi + sum_f log_var + sum_f mu^2*exp(-log_var)).

Strategy: data-parallel over batch (B=2048 -> 256 rows/core); weights
replicated. All layout work (transpose to f-major, SBUF-layout packing,
fp8 cast) and the O((B+C)F) elementwise weight prep happen on host; the
device does the O(B*F*C) GEMMs in fp8e4 DoubleRow mode (2 k-tiles per
matmul), accumulating fp32 in PSUM, then adds const[c] and DMAs out.

Two variants, picked per call:
 - general: outT = wcT*xT + wqT*x2T  (16 DoubleRow matmuls)
 - log_var constant across (c,f) (e.g. all zeros): wq[c,f] == wq0, so
   the quad term collapses to the rank-1 update ones[c] * (wq0*sum_f
   x[b,f]^2).  Host sends q[b] = wq0*sum_f x2 as one fp32 contraction
   row; the x2/wq chunk (half the input bytes) and its 8 matmuls
   disappear.

A few dummy matmuls on scratch SBUF run while the DMAs stream in,
keeping the PE busy so the HAM clock gate opens to 2.4 GHz.
"""
import sys

sys.path.insert(0, "/opt/trn_rl_repo")
import numpy as np
import concourse.bacc as bacc
import concourse.mybir as mybir
from concourse.tile import TileContext
from concourse.bass_utils import run_bass_kernel_spmd

B, C, F = 2048, 256, 1024
NCORES = 8
BSH = B // NCORES  # 256
KT = F // 128      # 8 k-tiles
LOG_2PI = float(np.log(2.0 * np.pi))
F32 = mybir.dt.float32
BF16 = mybir.dt.bfloat16
FP16 = mybir.dt.float16
FP8 = mybir.dt.float8e4
NPFP8 = mybir.dt.np(FP8)
DR = mybir.MatmulPerfMode.DoubleRow
WARM_MMS = 0
CHUNKS = (4, 2, 2)  # k-tiles per input chunk (need-ordered, serialized queue)

_CACHE = {}


def _build(rank1: bool):
    nc = bacc.Bacc("TRN2", target_bir_lowering=False, debug=False, num_devices=NCORES)
    # Host-packed fp8 chunks, f-major SBUF layout (dim1 = k-tile index).
    # Chunk j covers CHUNKS[j] k-tiles: first its xT k-tiles (r=b), then
    # the matching wcT k-tiles (r=c).  Chunks are DMA'd in need order on
    # one queue so matmuls start as soon as the first chunk lands and
    # only two matmuls remain after the last input byte.
    ach_d = [
        nc.dram_tensor(f"a{j}", [128, 2 * kk, 256], FP8, kind="ExternalInput").ap()
        for j, kk in enumerate(CHUNKS)
    ]
    if rank1:
        # aux row0: q[b] = wq0*sum_f x[b,f]^2 | ones[c]
        # aux row1: ones[b]                   | const[c]
        # K=2 rank-2 matmul adds both the quad term and const[c], so no
        # separate cst DMA or epilogue add is needed on this path.
        aux_d = nc.dram_tensor("aux", [2, 512], FP16, kind="ExternalInput").ap()
    else:
        # a2[p, i, r]: i 0..7 -> wqT k 0..7 (r=c), i 8..15 -> x2T k 0..7 (r=b)
        a2_d = nc.dram_tensor("a2", [128, 16, 256], FP8, kind="ExternalInput").ap()
        cst_d = nc.dram_tensor("cst", [128, 2], F32, kind="ExternalInput").ap()
    # out[p, 256m+b] = outT[128m+p, b]
    out_d = nc.dram_tensor("out", [128, 2 * BSH], F32, kind="ExternalOutput").ap()

    with TileContext(nc) as tc:
        with (
            tc.tile_pool(name="sb", bufs=1) as sb,
            tc.tile_pool(name="pg", bufs=2, space="PSUM") as pgp,
            tc.tile_pool(name="pw", bufs=1, space="PSUM") as pwp,
        ):
            # ---- PE warmup on scratch (result never read) while DMAs stream ----
            if WARM_MMS:
                junk = sb.tile([128, 512], BF16, tag="junk")
                wps = pwp.tile([128, 512], F32, tag="wps")
                nc.vector.memset(junk[:], 0)
                for _ in range(WARM_MMS):
                    nc.tensor.matmul(wps[:], junk[:, :128], junk[:], start=True, stop=True)

            # ---- DMA in (HWDGE via sync + scalar queues) ----
            # all x|wc chunks on the same (sync) queue: serialized in need
            # order, so early chunks complete while later ones stream.
            ach = [
                sb.tile([128, 2 * kk, 256], FP8, tag=f"a{j}", name=f"a{j}")
                for j, kk in enumerate(CHUNKS)
            ]
            for t_sb, t_d in zip(ach, ach_d):
                nc.sync.dma_start(out=t_sb[:], in_=t_d[:])
            if rank1:
                aux = sb.tile([2, 512], FP16, tag="aux")
                nc.scalar.dma_start(out=aux[:], in_=aux_d[:])
            else:
                a2 = sb.tile([128, 16, 256], FP8, tag="a2")
                nc.scalar.dma_start(out=a2[:, 0:8, :], in_=a2_d[:, 0:8, :])
                nc.scalar.dma_start(out=a2[:, 8:16, :], in_=a2_d[:, 8:16, :])
                cst = sb.tile([128, 2], F32, tag="cst")
                nc.scalar.dma_start(out=cst[:], in_=cst_d[:])

            # ---- GEMM: outT[c,b], fp8 DoubleRow (2 k-tiles per matmul) ----
            pg = [pgp.tile([128, BSH], F32, tag=f"pg{m}", name=f"pg{m}") for m in range(2)]
            started = [False, False]

            def mm(m, w_tile, w_i, a_tile, a_i, stop=False):
                nc.tensor.matmul(
                    pg[m][:],
                    w_tile[:, w_i:w_i + 2, m * 128:(m + 1) * 128],
                    a_tile[:, a_i:a_i + 2, :],
                    start=not started[m],
                    stop=stop,
                    perf_mode=DR,
                )
                started[m] = True

            def chunk_mms(j, stop=False):
                kk = CHUNKS[j]
                for t in range(kk // 2):
                    for m in range(2):
                        mm(m, ach[j], kk + 2 * t, ach[j], 2 * t,
                           stop=(stop and t == kk // 2 - 1))

            chunk_mms(0)
            if rank1:
                # += ones[c]*q[b] + const[c]*ones[b] (K=2).  aux's
                # descriptors drain behind chunk 0's at the SDMA engines,
                # so it lands just after — slot these here.
                for m in range(2):
                    nc.tensor.matmul(
                        pg[m][:],
                        aux[:, 256 + m * 128: 256 + m * 128 + 128],
                        aux[:, 0:256],
                        start=False,
                        stop=False,
                    )
            for j in range(1, len(CHUNKS)):
                chunk_mms(j, stop=(rank1 and j == len(CHUNKS) - 1))
            if not rank1:
                for m in range(2):       # wq * x2, k-tiles 0..7 (chunk a2)
                    for t in range(4):
                        mm(m, a2, 2 * t, a2, 8 + 2 * t, stop=(t == 3))

            # ---- epilogue: PSUM -> SBUF (+const on the general path), DMA out ----
            for m in range(2):
                osb = sb.tile([128, BSH], F32, tag=f"os{m}", name=f"os{m}")
                if rank1:
                    nc.vector.tensor_copy(osb[:], pg[m][:])
                else:
                    nc.vector.tensor_scalar_add(osb[:], pg[m][:], cst[:, m:m + 1])
                eng = nc.sync if m == 0 else nc.scalar
                eng.dma_start(out=out_d[:, m * BSH:(m + 1) * BSH], in_=osb[:])

    nc.compile()
    return nc


def get_nc(rank1=True):
    key = f"nc{int(bool(rank1))}"
    if key not in _CACHE:
        _CACHE[key] = _build(rank1)
    return _CACHE[key]


def _pack_fmajor(a):
    """(R, F) f32 -> [128, KT, R] fp8 with out[p, k, r] = a[r, 128k+p]."""
    R = a.shape[0]
    return a.reshape(R, KT, 128).transpose(2, 1, 0).astype(NPFP8)


def prepare_in_maps(x, mu, log_var, log_pi, force_general=False):
    x = np.asarray(x, dtype=np.float32)
    mu = np.asarray(mu, dtype=np.float32)
    lv = np.asarray(log_var, dtype=np.float32)
    lp = np.asarray(log_pi, dtype=np.float32)

    inv = np.exp(-lv)
    wc = mu * inv
    const = lp - 0.5 * (F * LOG_2PI + lv.sum(axis=1) + (mu * mu * inv).sum(axis=1))
    rank1 = bool(np.ptp(lv) == 0.0) and not force_general

    wcp = _pack_fmajor(wc)                      # [128, 8, 256]
    if not rank1:
        wqp = _pack_fmajor(-0.5 * inv)
        cst = np.ascontiguousarray(const.reshape(2, 128).T.astype(np.float32))
    else:
        wq0 = -0.5 * float(np.exp(-lv.flat[0]))

    maps = []
    for c in range(NCORES):
        xs = x[c * BSH:(c + 1) * BSH]
        xp = _pack_fmajor(xs)
        m = {}
        k0 = 0
        for j, kk in enumerate(CHUNKS):
            m[f"a{j}"] = np.ascontiguousarray(
                np.concatenate([xp[:, k0:k0 + kk], wcp[:, k0:k0 + kk]], axis=1))
            k0 += kk
        if rank1:
            aux = np.zeros((2, 512), dtype=np.float16)
            aux[0, 0:256] = (wq0 * (xs.astype(np.float64) ** 2).sum(axis=1)).astype(np.float16)
            aux[0, 256:512] = 1.0
            aux[1, 0:256] = 1.0
            aux[1, 256:512] = const.astype(np.float16)
            m["aux"] = aux
        else:
            m["a2"] = np.ascontiguousarray(
                np.concatenate([wqp, _pack_fmajor(xs * xs)], axis=1))
            m["cst"] = cst
        maps.append(m)
    return maps, rank1


def unpack_out(results):
    out = np.empty((B, C), dtype=np.float32)
    for c in range(NCORES):
        res = results[c]["out"].astype(np.float32)  # [128, 2*BSH] bf16 -> f32
        v = res.reshape(128, 2, BSH)
        out[c * BSH:(c + 1) * BSH, :] = v.transpose(2, 1, 0).reshape(BSH, C)
    return out


def kernel(x, mu, log_var, log_pi):
    in_maps, rank1 = prepare_in_maps(x, mu, log_var, log_pi)
    nc = get_nc(rank1)
    res = run_bass_kernel_spmd(nc, in_maps, list(range(NCORES)))
    return unpack_out(res.results)
